# revision 1
# baseline (speedup 1.0000x reference)
"""DBLoss (OHEM-masked BCE + masked L1 threshold loss) on 8 Trainium2 cores.

Shapes are hardcoded for the nn_DBLoss problem:
  outputs             [16, 3, 640, 640] f32
  gt_shrink_labels    [16, 640, 640]    f32
  gt_threshold_labels [16, 640, 640]    f32
Returns np.float32[4] = (loss_all, loss_shrink, loss_binary, loss_thresh).

Sharding: pure data parallel — 2 images per core, 8 cores. Each core computes
per-image partial sums (per-partition [128] vectors); the host reduces the
tiny partials and forms the masked means.

Math notes (device fast path):
 * OHEM: with neg_num == neg_total (i.e. 3*pos_num >= neg_total) the top-k
   threshold is the minimum negative score, so the selection mask is exactly
   all-ones for every valid image. The host verifies this condition per image
   (along with pos_num>0, neg_total>0) and falls back to an exact numpy
   implementation if any image needs a true top-k (cannot happen for the
   problem's uniform-random labels).
 * BCE with binarized target t and no sigmoid clipping reduces to
   softplus(x) - t*x; the host verifies |logits| < 16 so the 1e-7 clip in the
   reference is inactive.
 * threshold-loss mask (gt_t>0)|(gt_s>0): the device sums over all pixels;
   the host subtracts exact corrections for the (measure-zero) pixels where
   both labels are <= 0.
"""

import sys

import numpy as np

try:
    import concourse.bass as bass
except ImportError:  # stand-alone grading dir: fall back to known repo paths
    for _p in ("/root/.axon_site/_ro/trn_rl_repo", "/opt/trn_rl_repo"):
        if _p not in sys.path:
            sys.path.append(_p)
    import concourse.bass as bass

import concourse.tile as tile
from concourse import mybir
from concourse.bass_utils import run_bass_kernel_spmd

B, H, W = 16, 640, 640
N = H * W                    # 409600 pixels / image
P = 128                      # SBUF partitions
F = N // P                   # 3200 free elements / partition
NCORES = 8
BPC = B // NCORES            # 2 images per core
ALPHA, BETA = 1.0, 10.0
F32 = mybir.dt.float32

_CACHED_NC = None


def build_nc() -> "bass.Bass":
    """Per-core raw-bass program.

    Per image: 5 HWDGE channel loads, 7 ACT table ops (exp/ln set only),
    4 big DVE ops; per-partition partial sums in one output tile.

    Raw bass (no TileContext): this walrus build encodes at most ONE attached
    sync-wait per TPB instruction and Tile's kernel-tail drain needs ~10, so
    all cross-engine ordering uses standalone wait_ge instructions
    (EventSemaphore ops, which codegen fine) with explicit semaphores.

    Load order is tuned so ACT (the busiest engine at ~41.4 us of table ops)
    starts after the first 1.6 MB load and never stalls long, and so the
    last-arriving tensors gate the least trailing work:
      tm0 g0 s0 bn0 tm1 gt0 g1 s1 gt1 bn1

    Semaphores: one per input DMA (+16 on completion), sa = ACT op counter
    (then_inc fires on write-ack, so sa>=k also guards same-engine RAW/WAW
    on ACT outputs), sv = DVE op counter, sc = bias-constant memset done,
    dout = output DMA completion. DVE clears every semaphore at the end so
    repeated executions of the loaded NEFF start from zero.
    """
    nc = bass.Bass(dynamic_dma_scratch_size=2048, enable_partition_id=False,
                   monotonic_sem_count=0)
    outs = nc.dram_tensor("outs", [BPC, 3, N], F32, kind="ExternalInput")
    gts = nc.dram_tensor("gts", [BPC, N], F32, kind="ExternalInput")
    gtt = nc.dram_tensor("gtt", [BPC, N], F32, kind="ExternalInput")
    # columns per image b: [2b]=sum softplus(shrink), [2b+1]=sum softplus(bin)
    # then [4+3b]=sum t*shrink, [5+3b]=sum t*bin, [6+3b]=sum|sig-gt|
    part = nc.dram_tensor("part", [P, 12], F32, kind="ExternalOutput")

    ag = mybir.AluOpType.is_gt
    mul = mybir.AluOpType.mult
    sub = mybir.AluOpType.subtract
    fexp = mybir.ActivationFunctionType.Exp
    fln = mybir.ActivationFunctionType.Ln
    X = mybir.AxisListType.X
    add = mybir.AluOpType.add

    from contextlib import ExitStack
    ctx = ExitStack()
    with ctx:
        sb = lambda nm, shape: ctx.enter_context(nc.sbuf_tensor(nm, shape, F32))
        sem = lambda nm: ctx.enter_context(nc.semaphore(name=nm))
        tm = [sb("tm_0", [P, F]), sb("tm_1", [P, F])]
        s = [sb("s_0", [P, F]), sb("s_1", [P, F])]
        bn = [sb("bn_0", [P, F]), sb("bn_1", [P, F])]
        g = [sb("g_0", [P, F]), sb("g_1", [P, F])]
        gt = [sb("gt_0", [P, F]), sb("gt_1", [P, F])]
        u = [sb("u_0", [P, F]), sb("u_1", [P, F])]
        eu, tr = sb("eu", [P, F]), sb("tr", [P, F])
        po = sb("po", [P, 12])
        bias1 = sb("bias1", [P, 1])
        dtm = [sem("dtm0"), sem("dtm1")]
        ds = [sem("ds0"), sem("ds1")]
        dbn = [sem("dbn0"), sem("dbn1")]
        dbnb = sem("dbnb")
        dg = [sem("dg0"), sem("dg1")]
        dgt = [sem("dgt0"), sem("dgt1")]
        dout, sa, sv, sc = (sem(nm) for nm in ("dout", "sa", "sv", "sc"))
        all_sems = (dtm + ds + dbn + dg + dgt + [dbnb, dout, sa, sv, sc])
        block = ctx.enter_context(nc.Block(no_gpsimd_drain=True))

        pf = lambda t: t.rearrange("(p f) -> p f", p=P)

        @block.sync
        def _(sync):
            loads = [
                (tm[0], outs[0, 1], dtm[0]),
                (s[0], outs[0, 0], ds[0]),
                (g[0], gts[0], dg[0]),
                (bn[0], outs[0, 2], dbn[0]),
                (tm[1], outs[1, 1], dtm[1]),
                (gt[0], gtt[0], dgt[0]),
                (s[1], outs[1, 0], ds[1]),
                (g[1], gts[1], dg[1]),
                (gt[1], gtt[1], dgt[1]),
            ]
            for dst, src, dsem in loads:
                sync.dma_start(out=dst[:, :], in_=pf(src)).then_inc(dsem, 16)
            h = F // 2
            bn1f = pf(outs[1, 2])
            sync.dma_start(out=bn[1][:, :h], in_=bn1f[:, :h]).then_inc(dbn[1], 16)
            sync.dma_start(out=bn[1][:, h:], in_=bn1f[:, h:]).then_inc(dbnb, 16)
            sync.wait_ge(sa, 7 * BPC + 2)
            sync.wait_ge(sv, 4 * BPC + 1)
            sync.dma_start(out=part[:, :], in_=po[:, :]).then_inc(dout, 16)
            for semh in all_sems:
                if semh is not dout:
                    sync.sem_clear(semh)
            sync.wait_ge(dout, 16)
            sync.sem_clear(dout)

        @block.scalar
        def _(scalar):
            sa_n = 0

            def act(out, in_, func, wait_prev=True, **kw):
                # previous-op write-ack rides as the instruction's single
                # attached sync-wait (walrus allows exactly one)
                nonlocal sa_n
                inst = nc.scalar.activation(out=out, in_=in_, func=func,
                                            **kw).then_inc(sa, 1)
                if wait_prev and sa_n >= 1:
                    inst.wait_op(sa, sa_n, "sem-ge")
                sa_n += 1

            for b in range(BPC):
                # sigmoid(tm) = exp(-ln(1 + exp(-tm))) in place in u[b]
                scalar.wait_ge(dtm[b], 16)
                act(u[b][:, :], tm[b][:, :], fexp, wait_prev=False, scale=-1.0)
                if b == 0:
                    scalar.wait_ge(sc, 1)
                act(u[b][:, :], u[b][:, :], fln, bias=bias1[:, :])
                act(u[b][:, :], u[b][:, :], fexp, scale=-1.0)
                # BCE softplus sums: ln(1 + exp(x)), accumulated per partition
                scalar.wait_ge(ds[b], 16)
                act(eu[:, :], s[b][:, :], fexp)
                act(eu[:, :], eu[:, :], fln, bias=bias1[:, :],
                    accum_out=po[:, 2 * b : 2 * b + 1])
                if b == 0:
                    scalar.wait_ge(dbn[b], 16)
                    act(eu[:, :], bn[b][:, :], fexp)
                    act(eu[:, :], eu[:, :], fln, bias=bias1[:, :],
                        accum_out=po[:, 1:2])
                else:
                    # bn1 arrives last: process halves as they land
                    h = F // 2
                    scalar.wait_ge(dbn[b], 16)
                    act(eu[:, :h], bn[b][:, :h], fexp)
                    act(eu[:, :h], eu[:, :h], fln, bias=bias1[:, :],
                        accum_out=po[:, 3:4])
                    scalar.wait_ge(dbnb, 16)
                    act(eu[:, h:], bn[b][:, h:], fexp)
                    act(eu[:, h:], eu[:, h:], fln, bias=bias1[:, :],
                        accum_out=po[:, 4:5])
            assert sa_n == 7 * BPC + 2

        @block.vector
        def _(vector):
            nc.vector.memset(bias1[:, :], 1.0).then_inc(sc, 1)
            sv_n = 0

            def stt_sum(b, which, half=None):
                # sum (g>0.5)*x; writes (a slice of) tr
                nonlocal sv_n
                h = F // 2
                cols = {(0, "s"): 5, (0, "bn"): 6, (1, "s"): 8,
                        (1, "bn", 0): 9, (1, "bn", 1): 10}
                if half is None:
                    col = cols[(b, which)]
                    sl = slice(None)
                    dsem = ds[b] if which == "s" else dbn[b]
                else:
                    col = cols[(b, which, half)]
                    sl = slice(0, h) if half == 0 else slice(h, F)
                    dsem = dbn[b] if half == 0 else dbnb
                x = s if which == "s" else bn
                vector.wait_ge(dg[b], 16)
                vector.wait_ge(dsem, 16)
                inst = nc.vector.scalar_tensor_tensor(
                    out=tr[:, sl], in0=g[b][:, sl], scalar=0.5,
                    in1=x[b][:, sl], op0=ag, op1=mul,
                    accum_out=po[:, col : col + 1],
                ).then_inc(sv, 1)
                if sv_n >= 1:
                    inst.wait_op(sv, sv_n, "sem-ge")  # tr write-ack of prev op
                sv_n += 1

            def l1_pair(b):
                # |sigmoid - gt| summed: subtract in place into gt, abs-reduce
                nonlocal sv_n
                vector.wait_ge(sa, 7 * b + 3)   # sigmoid chain done
                vector.wait_ge(dgt[b], 16)
                nc.vector.tensor_tensor(
                    out=gt[b][:, :], in0=u[b][:, :], in1=gt[b][:, :], op=sub
                ).then_inc(sv, 1)
                sv_n += 1
                nc.vector.tensor_reduce(
                    out=po[:, 7 + 4 * b : 8 + 4 * b], in_=gt[b][:, :],
                    axis=X, op=add, apply_absolute_value=True,
                ).then_inc(sv, 1).wait_op(sv, sv_n, "sem-ge")
                sv_n += 1

            # image 0: bn arrives before gt; image 1: bn arrives last, halved
            stt_sum(0, "s")
            stt_sum(0, "bn")
            l1_pair(0)
            stt_sum(1, "s")
            l1_pair(1)
            stt_sum(1, "bn", half=0)
            stt_sum(1, "bn", half=1)
            assert sv_n == 4 * BPC + 1

    return nc


def _numpy_reference(outputs, gt_shrink_labels, gt_threshold_labels):
    """Exact fallback for inputs outside the fast-path regime."""
    OHEM_RATIO, EPS = 3, 1e-7

    def sigmoid(x):
        return 1.0 / (1.0 + np.exp(-x))

    shrink, thresh, binary = outputs[:, 0], outputs[:, 1], outputs[:, 2]
    b = outputs.shape[0]
    flat_s = shrink.reshape(b, -1)
    flat_pos = (gt_shrink_labels > 0.5).reshape(b, -1)
    n = flat_s.shape[1]
    pos_num = flat_pos.sum(axis=1)
    neg_total = n - pos_num
    neg_num = np.minimum(pos_num * OHEM_RATIO, neg_total)
    neg_scores = np.where(flat_pos, -np.inf, flat_s)
    sorted_desc = -np.sort(-neg_scores, axis=1)
    idx = np.clip(neg_num - 1, 0, n - 1).astype(np.int64)
    thr = np.take_along_axis(sorted_desc, idx[:, None], axis=1)
    mask = (flat_s >= thr) | flat_pos
    valid = (pos_num > 0) & (neg_num > 0)
    mask = (mask & valid[:, None]).reshape(shrink.shape).astype(np.float32)

    def masked_bce(logits, target, m):
        p = np.clip(sigmoid(logits), EPS, 1.0 - EPS)
        t = (target > 0.5).astype(np.float32)
        per_px = -(t * np.log(p) + (1.0 - t) * np.log(1.0 - p))
        denom = m.sum()
        return float(per_px.flatten() @ m.flatten() / max(denom, 1.0)) if denom > 0 else 0.0

    loss_shrink = masked_bce(shrink, gt_shrink_labels, mask)
    loss_binary = masked_bce(binary, gt_shrink_labels, mask)
    m2 = ((gt_threshold_labels > 0) | (gt_shrink_labels > 0)).astype(np.float32)
    denom2 = m2.sum()
    l1 = np.abs(sigmoid(thresh) - gt_threshold_labels).flatten() @ m2.flatten()
    loss_thresh = float(l1 / max(denom2, 1.0)) if denom2 > 0 else 0.0
    loss_all = loss_shrink + ALPHA * loss_binary + BETA * loss_thresh
    return np.array([loss_all, loss_shrink, loss_binary, loss_thresh], np.float32)


def kernel(outputs, gt_shrink_labels, gt_threshold_labels, _trace=False):
    global _CACHED_NC
    outputs = np.ascontiguousarray(np.asarray(outputs, dtype=np.float32))
    gts = np.ascontiguousarray(np.asarray(gt_shrink_labels, dtype=np.float32))
    gtt = np.ascontiguousarray(np.asarray(gt_threshold_labels, dtype=np.float32))

    # ---- host-side regime checks (exactness guards for the fast path) ----
    pos_num = (gts > 0.5).reshape(B, -1).sum(axis=1)
    neg_total = N - pos_num
    neg_num = np.minimum(3 * pos_num, neg_total)
    valid = (pos_num > 0) & (neg_num > 0)
    needs_topk = valid & (3 * pos_num < neg_total)
    clip_active = max(
        float(np.abs(outputs[:, 0]).max()), float(np.abs(outputs[:, 2]).max())
    ) >= 16.0
    if needs_topk.any() or clip_active:
        return _numpy_reference(outputs, gts, gtt)

    if _CACHED_NC is None:
        _CACHED_NC = build_nc()
    nc = _CACHED_NC

    in_maps = []
    for c in range(NCORES):
        sl = slice(c * BPC, (c + 1) * BPC)
        in_maps.append({
            "outs": outputs[sl].reshape(BPC, 3, N),
            "gts": gts[sl].reshape(BPC, N),
            "gtt": gtt[sl].reshape(BPC, N),
        })
    res = run_bass_kernel_spmd(
        nc, in_maps, core_ids=list(range(NCORES)), trace=_trace
    )

    # ---- host combine: per-image sums from per-partition partials ----
    sp_s = np.empty(B); sp_b = np.empty(B); ts = np.empty(B); tb = np.empty(B)
    l1 = np.empty(B)
    for c in range(NCORES):
        po = res.results[c]["part"].astype(np.float64).sum(axis=0)
        i0, i1 = c * BPC, c * BPC + 1
        sp_s[i0], sp_b[i0] = po[0], po[1]
        sp_s[i1], sp_b[i1] = po[2], po[3] + po[4]
        ts[i0], tb[i0], l1[i0] = po[5], po[6], po[7]
        ts[i1], tb[i1], l1[i1] = po[8], po[9] + po[10], po[11]

    cnt = float(N * valid.sum())
    num_s = float(((sp_s - ts) * valid).sum())
    num_b = float(((sp_b - tb) * valid).sum())
    loss_shrink = num_s / max(cnt, 1.0) if cnt > 0 else 0.0
    loss_binary = num_b / max(cnt, 1.0) if cnt > 0 else 0.0

    # threshold-loss mask corrections for pixels where both labels <= 0
    zz = (gtt <= 0) & (gts <= 0)
    cnt2 = float(B * N - zz.sum())
    l1_tot = float(l1.sum())
    if zz.any():
        tmz = outputs[:, 1][zz]
        l1_tot -= float(np.abs(1.0 / (1.0 + np.exp(-tmz)) - gtt[zz]).sum())
    loss_thresh = l1_tot / max(cnt2, 1.0) if cnt2 > 0 else 0.0

    loss_all = loss_shrink + ALPHA * loss_binary + BETA * loss_thresh
    out = np.array([loss_all, loss_shrink, loss_binary, loss_thresh], np.float32)
    if _trace:
        return out, res
    return out



# revision 8
# speedup vs baseline: 1.3829x; 1.3829x over previous
"""DBLoss (OHEM-masked BCE + masked L1 threshold loss) on 8 Trainium2 cores.

Shapes are hardcoded for the nn_DBLoss problem:
  outputs             [16, 3, 640, 640] f32
  gt_shrink_labels    [16, 640, 640]    f32
  gt_threshold_labels [16, 640, 640]    f32
Returns np.float32[4] = (loss_all, loss_shrink, loss_binary, loss_thresh).

Sharding: pure data parallel - 2 images per core, 8 cores. Each core computes
per-image partial sums (per-partition [128] vectors); the host reduces the
tiny partials and forms the masked means.

v4 design (vs the exp/ln-chain baseline):
 * Host stages all five input planes as fp16 (memory regime: halves HBM
   traffic; the losses are means over 409600 pixels, so the ~5e-4 relative
   input rounding averages out far below the 2e-2 tolerance).
 * ACT: native Sigmoid table for the two tm planes (this platform's PWP has
   no softplus table), then one switch to natural_log_exp_and_others for the
   four BCE softplus sums as exp then ln(1+u)-accumulate: 10 table passes +
   1 on-path table load (~31us busy) instead of the baseline's 14 passes
   (~41us). A dummy sigmoid at t=0 pulls the first table load off the
   critical path, and sigmoids run first so the L1-term DVE work clears
   early while the BCE ln-accumulates ARE the final output (no DVE behind
   them).
 * All DVE work is fp16 2x-mode fused scalar_tensor_tensor with f32
   accumulators: masked sums as (g > 0.5) * x, and the L1 term via
   sum|sig-gt| = sum max(sig,gt) - sum min(sig,gt)  (host subtracts),
   which needs no subtract / abs / tensor_reduce ops at all.
 * Same fast-path math as the baseline: with neg_num == neg_total the OHEM
   mask is all-ones for every valid image (host verifies, exact numpy
   fallback otherwise); BCE reduces to softplus(x) - t*x; threshold-loss mask
   corrections for (gt_t<=0)&(gt_s<=0) pixels are applied on the host.
"""

import sys

import numpy as np

try:
    import concourse.bass as bass
except ImportError:  # stand-alone grading dir: fall back to known repo paths
    for _p in ("/root/.axon_site/_ro/trn_rl_repo", "/opt/trn_rl_repo"):
        if _p not in sys.path:
            sys.path.append(_p)
    import concourse.bass as bass

from concourse import mybir
from concourse.bass_utils import run_bass_kernel_spmd

B, H, W = 16, 640, 640
N = H * W                    # 409600 pixels / image
P = 128                      # SBUF partitions
F = N // P                   # 3200 free elements / partition
NCORES = 8
BPC = B // NCORES            # 2 images per core
ALPHA, BETA = 1.0, 10.0
F16 = mybir.dt.float16
F32 = mybir.dt.float32

# DMA (= DRAM slice) order of the 10 per-core [128, F] fp16 planes.
# ACT consumes tm first (sigmoid set), then s/bn (softplus set, after the
# one table switch); DVE consumes gt early (L1 max/min sums) and g/s/bn
# later (masked sums). This order keeps both engines fed while DMA streams.
SLOT_TM0, SLOT_TM1, SLOT_GT0, SLOT_S0, SLOT_GT1 = 0, 1, 2, 3, 4
SLOT_BN0, SLOT_G0, SLOT_S1, SLOT_G1, SLOT_BN1 = 5, 6, 7, 8, 9

_CACHED_NC = None


def build_nc() -> "bass.Bass":
    """Per-core raw-bass program.

    po [128, 12] f32 accumulator columns:
      0: sum softplus(s0)   1: sum softplus(bn0)
      2: sum softplus(s1)   3: sum softplus(bn1)
      4: sum t0*s0  5: sum t0*bn0  6: sum t1*s1  7: sum t1*bn1
      8: sum max(sig0,gt0)  9: sum min(sig0,gt0)
     10: sum max(sig1,gt1) 11: sum min(sig1,gt1)

    Raw bass (no TileContext): cross-engine ordering uses standalone wait_ge
    instructions with explicit semaphores; every data op carries then_inc on
    its own engine counter (fires on write-ack, so waiting on the counter
    also guards RAW across engines).
    """
    nc = bass.Bass(dynamic_dma_scratch_size=2048, enable_partition_id=False,
                   monotonic_sem_count=0)
    data = nc.dram_tensor("data", [10, P, F], F16, kind="ExternalInput")
    part = nc.dram_tensor("part", [P, 12], F32, kind="ExternalOutput")

    EXP = mybir.ActivationFunctionType.Exp
    LN = mybir.ActivationFunctionType.Ln
    SG = mybir.ActivationFunctionType.Sigmoid
    ag = mybir.AluOpType.is_gt
    mul = mybir.AluOpType.mult
    mx = mybir.AluOpType.max
    mn = mybir.AluOpType.min

    from contextlib import ExitStack
    ctx = ExitStack()
    with ctx:
        sb = lambda nm, shape, dt: ctx.enter_context(nc.sbuf_tensor(nm, shape, dt))
        sem = lambda nm: ctx.enter_context(nc.semaphore(name=nm))
        T = [sb(f"t{i}", [P, F], F16) for i in range(10)]
        E = [sb("e0", [P, F], F32), sb("e1", [P, F], F32)]
        w = sb("w", [P, F], F16)
        po = sb("po", [P, 12], F32)
        d = [sem(f"d{i}") for i in range(10)]
        sa, sv, dout = sem("sa"), sem("sv"), sem("dout")
        block = ctx.enter_context(nc.Block(no_gpsimd_drain=True))

        @block.sync
        def _(sync):
            for i in range(10):
                sync.dma_start(out=T[i][:, :], in_=data[i]).then_inc(d[i], 16)
            sync.wait_ge(sa, 10)
            sync.wait_ge(sv, 8)
            sync.dma_start(out=part[:, :], in_=po[:, :]).then_inc(dout, 16)
            for s_ in d + [sa, sv]:
                sync.sem_clear(s_)
            sync.wait_ge(dout, 16)
            sync.sem_clear(dout)

        @block.scalar
        def _(scalar):
            # Dummy 1-column sigmoid: walrus places the sigmoid-set
            # ACT_TABLE_LOAD before it, so the load overlaps the first DMA.
            nc.scalar.activation(out=w[:, 0:1], in_=w[:, 0:1], func=SG)
            # sigmoids in place on the tm tiles (sa=1, 2)
            for ti in (SLOT_TM0, SLOT_TM1):
                scalar.wait_ge(d[ti], 16)
                nc.scalar.activation(
                    out=T[ti][:, :], in_=T[ti][:, :], func=SG
                ).then_inc(sa, 1)
            # table switch to natural_log_exp happens before the first Exp;
            # per BCE plane: u = exp(x), then ln(1 + u) accumulated (sa=3..10)
            for k, (ti, col) in enumerate(
                [(SLOT_S0, 0), (SLOT_BN0, 1), (SLOT_S1, 2), (SLOT_BN1, 3)]
            ):
                e = E[k % 2]
                scalar.wait_ge(d[ti], 16)
                nc.scalar.activation(
                    out=e[:, :], in_=T[ti][:, :], func=EXP,
                ).then_inc(sa, 1)
                nc.scalar.activation(
                    out=e[:, :], in_=e[:, :], func=LN, bias=1.0,
                    accum_out=po[:, col : col + 1],
                ).then_inc(sa, 1)

        @block.vector
        def _(vector):
            def stt(i0, i1, op0, op1, col, scalar=1.0):
                nc.vector.scalar_tensor_tensor(
                    out=w[:, :], in0=T[i0][:, :], scalar=scalar,
                    in1=T[i1][:, :], op0=op0, op1=op1,
                    accum_out=po[:, col : col + 1],
                ).then_inc(sv, 1)

            # L1 term, image 0: sum max/min(sig0, gt0)
            vector.wait_ge(sa, 1)            # sigmoid(tm0) write-ack
            vector.wait_ge(d[SLOT_GT0], 16)
            stt(SLOT_TM0, SLOT_GT0, mul, mx, 8)
            stt(SLOT_TM0, SLOT_GT0, mul, mn, 9)
            # L1 term, image 1
            vector.wait_ge(sa, 2)
            vector.wait_ge(d[SLOT_GT1], 16)
            stt(SLOT_TM1, SLOT_GT1, mul, mx, 10)
            stt(SLOT_TM1, SLOT_GT1, mul, mn, 11)
            # masked sums: (g > 0.5) * x
            vector.wait_ge(d[SLOT_G0], 16)
            vector.wait_ge(d[SLOT_S0], 16)
            stt(SLOT_G0, SLOT_S0, ag, mul, 4, scalar=0.5)
            vector.wait_ge(d[SLOT_BN0], 16)
            stt(SLOT_G0, SLOT_BN0, ag, mul, 5, scalar=0.5)
            vector.wait_ge(d[SLOT_G1], 16)
            vector.wait_ge(d[SLOT_S1], 16)
            stt(SLOT_G1, SLOT_S1, ag, mul, 6, scalar=0.5)
            vector.wait_ge(d[SLOT_BN1], 16)
            stt(SLOT_G1, SLOT_BN1, ag, mul, 7, scalar=0.5)

    return nc


def _numpy_reference(outputs, gt_shrink_labels, gt_threshold_labels):
    """Exact fallback for inputs outside the fast-path regime."""
    OHEM_RATIO, EPS = 3, 1e-7

    def sigmoid(x):
        return 1.0 / (1.0 + np.exp(-x))

    shrink, thresh, binary = outputs[:, 0], outputs[:, 1], outputs[:, 2]
    b = outputs.shape[0]
    flat_s = shrink.reshape(b, -1)
    flat_pos = (gt_shrink_labels > 0.5).reshape(b, -1)
    n = flat_s.shape[1]
    pos_num = flat_pos.sum(axis=1)
    neg_total = n - pos_num
    neg_num = np.minimum(pos_num * OHEM_RATIO, neg_total)
    neg_scores = np.where(flat_pos, -np.inf, flat_s)
    sorted_desc = -np.sort(-neg_scores, axis=1)
    idx = np.clip(neg_num - 1, 0, n - 1).astype(np.int64)
    thr = np.take_along_axis(sorted_desc, idx[:, None], axis=1)
    mask = (flat_s >= thr) | flat_pos
    valid = (pos_num > 0) & (neg_num > 0)
    mask = (mask & valid[:, None]).reshape(shrink.shape).astype(np.float32)

    def masked_bce(logits, target, m):
        p = np.clip(sigmoid(logits), EPS, 1.0 - EPS)
        t = (target > 0.5).astype(np.float32)
        per_px = -(t * np.log(p) + (1.0 - t) * np.log(1.0 - p))
        denom = m.sum()
        return float(per_px.flatten() @ m.flatten() / max(denom, 1.0)) if denom > 0 else 0.0

    loss_shrink = masked_bce(shrink, gt_shrink_labels, mask)
    loss_binary = masked_bce(binary, gt_shrink_labels, mask)
    m2 = ((gt_threshold_labels > 0) | (gt_shrink_labels > 0)).astype(np.float32)
    denom2 = m2.sum()
    l1 = np.abs(sigmoid(thresh) - gt_threshold_labels).flatten() @ m2.flatten()
    loss_thresh = float(l1 / max(denom2, 1.0)) if denom2 > 0 else 0.0
    loss_all = loss_shrink + ALPHA * loss_binary + BETA * loss_thresh
    return np.array([loss_all, loss_shrink, loss_binary, loss_thresh], np.float32)


def kernel(outputs, gt_shrink_labels, gt_threshold_labels, _trace=False):
    global _CACHED_NC
    outputs = np.ascontiguousarray(np.asarray(outputs, dtype=np.float32))
    gts = np.ascontiguousarray(np.asarray(gt_shrink_labels, dtype=np.float32))
    gtt = np.ascontiguousarray(np.asarray(gt_threshold_labels, dtype=np.float32))

    # ---- host-side regime checks (exactness guards for the fast path) ----
    pos_num = (gts > 0.5).reshape(B, -1).sum(axis=1)
    neg_total = N - pos_num
    neg_num = np.minimum(3 * pos_num, neg_total)
    valid = (pos_num > 0) & (neg_num > 0)
    needs_topk = valid & (3 * pos_num < neg_total)
    clip_active = max(
        float(np.abs(outputs[:, 0]).max()), float(np.abs(outputs[:, 2]).max())
    ) >= 16.0
    if needs_topk.any() or clip_active:
        return _numpy_reference(outputs, gts, gtt)

    if _CACHED_NC is None:
        _CACHED_NC = build_nc()
    nc = _CACHED_NC

    # ---- fp16 staging, packed per core in DMA order ----
    big = np.empty((NCORES, 10, P, F), np.float16)
    for c in range(NCORES):
        i0, i1 = c * BPC, c * BPC + 1
        big[c, SLOT_TM0] = outputs[i0, 1].reshape(P, F)
        big[c, SLOT_TM1] = outputs[i1, 1].reshape(P, F)
        big[c, SLOT_GT0] = gtt[i0].reshape(P, F)
        big[c, SLOT_S0] = outputs[i0, 0].reshape(P, F)
        big[c, SLOT_GT1] = gtt[i1].reshape(P, F)
        big[c, SLOT_BN0] = outputs[i0, 2].reshape(P, F)
        big[c, SLOT_G0] = gts[i0].reshape(P, F)
        big[c, SLOT_S1] = outputs[i1, 0].reshape(P, F)
        big[c, SLOT_G1] = gts[i1].reshape(P, F)
        big[c, SLOT_BN1] = outputs[i1, 2].reshape(P, F)

    in_maps = [{"data": big[c]} for c in range(NCORES)]
    res = run_bass_kernel_spmd(
        nc, in_maps, core_ids=list(range(NCORES)), trace=_trace
    )

    # ---- host combine: per-image sums from per-partition partials ----
    sp_s = np.empty(B); sp_b = np.empty(B); ts = np.empty(B); tb = np.empty(B)
    l1 = np.empty(B)
    for c in range(NCORES):
        po = res.results[c]["part"].astype(np.float64).sum(axis=0)
        i0, i1 = c * BPC, c * BPC + 1
        sp_s[i0], sp_b[i0] = po[0], po[1]
        sp_s[i1], sp_b[i1] = po[2], po[3]
        ts[i0], tb[i0] = po[4], po[5]
        ts[i1], tb[i1] = po[6], po[7]
        l1[i0] = po[8] - po[9]
        l1[i1] = po[10] - po[11]

    cnt = float(N * valid.sum())
    num_s = float(((sp_s - ts) * valid).sum())
    num_b = float(((sp_b - tb) * valid).sum())
    loss_shrink = num_s / max(cnt, 1.0) if cnt > 0 else 0.0
    loss_binary = num_b / max(cnt, 1.0) if cnt > 0 else 0.0

    # threshold-loss mask corrections for pixels where both labels <= 0
    zz = (gtt <= 0) & (gts <= 0)
    cnt2 = float(B * N - zz.sum())
    l1_tot = float(l1.sum())
    if zz.any():
        tmz = outputs[:, 1][zz]
        l1_tot -= float(np.abs(1.0 / (1.0 + np.exp(-tmz)) - gtt[zz]).sum())
    loss_thresh = l1_tot / max(cnt2, 1.0) if cnt2 > 0 else 0.0

    loss_all = loss_shrink + ALPHA * loss_binary + BETA * loss_thresh
    out = np.array([loss_all, loss_shrink, loss_binary, loss_thresh], np.float32)
    if _trace:
        return out, res
    return out
